# revision 14
# baseline (speedup 1.0000x reference)
"""Trainium2 Bass kernel for banded (local-window) multi-head attention.

Reference computes, for x (B=2, T=2048, C=512):
    qkv = x @ w_attn + b_attn ; split into per-head q, k, v (H=8, hs=64)
    scores = q @ k^T / sqrt(hs), masked to the band j in [i-16, i]
    attn = softmax(scores)            -> (B, H, T, T), zero outside band
    y    = (attn @ v) @ w_proj + b_proj
returns (y, attn).

Sharding: data-parallel over (batch, token-quarter) -> 8 cores, each
handling 512 consecutive tokens of one batch with a 16-token halo of
x for the banded keys/values.  Each core computes the full qkv
projection for its shard, the banded softmax, and its slice of the
projected output.  The device emits attn as dense (128 x 144) window
tiles (zero outside the band by construction); the host scatters them
into the big, mostly-zero (B, H, T, T) output.
"""

import math

import numpy as np

import concourse.mybir as mybir
import concourse.tile as tile
from concourse import bacc
from concourse.bass_utils import run_bass_kernel_spmd
from concourse.masks import make_identity

F32 = mybir.dt.float32

B, T, C = 2, 2048, 512
H, HS = 8, 64
W = 16              # attention window (j in [i-16, i])
NC = 8              # cores
TS = T * B // NC    # tokens per shard = 512
LT = TS + W         # local tokens incl. halo = 528
NKT = TS // 128     # 128-token q tiles per shard = 4
WIN = 128 + W       # key window per q tile = 144
NCI = C // 128      # contraction chunks for projections = 4
NPAIR = H // 2      # head pairs = 4
TOK_SIZES = (128, 128, 128, 128, W)


def build_program():
    nc = bacc.Bacc("TRN2", target_bir_lowering=False, debug=False)

    xt_d = nc.dram_tensor("xT", [C, LT], F32, kind="ExternalInput").ap()
    wa_d = nc.dram_tensor("w_attn", [C, 3 * C], F32, kind="ExternalInput").ap()
    bqk_d = nc.dram_tensor("b_qk", [128, 8], F32, kind="ExternalInput").ap()
    bv_d = nc.dram_tensor("b_v", [1, C], F32, kind="ExternalInput").ap()
    wp_d = nc.dram_tensor("w_proj", [C, C], F32, kind="ExternalInput").ap()
    bp_d = nc.dram_tensor("b_proj", [1, C], F32, kind="ExternalInput").ap()
    mk_d = nc.dram_tensor("masks", [2, 128, 2 * WIN], F32, kind="ExternalInput").ap()

    y_d = nc.dram_tensor("y", [TS, C], F32, kind="ExternalOutput").ap()
    at_d = nc.dram_tensor("attn_t", [NKT, H, 128, WIN], F32, kind="ExternalOutput").ap()

    with tile.TileContext(nc) as tc:
        with (
            tc.tile_pool(name="persist", bufs=1) as pp,
            tc.tile_pool(name="work", bufs=2) as wk,
        ):
            # ---- constants / parameters ----
            ident = pp.tile([128, 128], F32, tag="ident")
            make_identity(nc, ident[:])
            ones = pp.tile([1, LT], F32, tag="ones")
            nc.gpsimd.memset(ones[:], 1.0)

            w_sb = []
            for ci in range(NCI):
                t = pp.tile([128, 3 * C], F32, tag=f"w{ci}")
                nc.sync.dma_start(out=t[:], in_=wa_d[128 * ci:128 * (ci + 1), :])
                w_sb.append(t)
            wp_sb = []
            for pr in range(NPAIR):
                t = pp.tile([128, C], F32, tag=f"wp{pr}")
                nc.sync.dma_start(out=t[:], in_=wp_d[128 * pr:128 * (pr + 1), :])
                wp_sb.append(t)
            bqk = pp.tile([128, 8], F32, tag="bqk")
            nc.sync.dma_start(out=bqk[:], in_=bqk_d[:])
            bv = pp.tile([1, C], F32, tag="bv")
            nc.sync.dma_start(out=bv[:], in_=bv_d[:])
            bp = pp.tile([1, C], F32, tag="bp")
            nc.sync.dma_start(out=bp[:], in_=bp_d[:])
            mk_sb = []
            for v in range(2):
                t = pp.tile([128, 2 * WIN], F32, tag=f"mask{v}")
                nc.sync.dma_start(out=t[:], in_=mk_d[v])
                mk_sb.append(t)

            xT_sb = [pp.tile([128, LT], F32, tag=f"xT{ci}", name=f"xT{ci}")
                     for ci in range(NCI)]
            # chunk cc in 0..3 -> q head-pair cc ; cc in 4..7 -> k head-pair cc-4
            qkT_sb = [pp.tile([128, LT], F32, tag=f"qkT{cc}", name=f"qkT{cc}")
                      for cc in range(8)]
            v_sb = [pp.tile([n, C], F32, tag=f"v{vt}", name=f"v{vt}")
                    for vt, n in enumerate(TOK_SIZES)]

            # ---- phase 1: load xT, project qkT and v ----
            with (
                tc.tile_pool(name="ps_proj", bufs=2, space="PSUM") as ps_proj,
            ):
                for ci in range(NCI):
                    nc.sync.dma_start(
                        out=xT_sb[ci][:],
                        in_=xt_d[128 * ci:128 * (ci + 1), :],
                    )

                for cc in range(8):
                    ps = ps_proj.tile([128, LT], F32, tag="qkvps")
                    for n0, nn in ((0, 512), (512, LT - 512)):
                        for ci in range(NCI):
                            nc.tensor.matmul(
                                ps[:, n0:n0 + nn],
                                w_sb[ci][:, 128 * cc:128 * (cc + 1)],
                                xT_sb[ci][:, n0:n0 + nn],
                                start=(ci == 0),
                                stop=(ci == NCI - 1),
                            )
                    nc.scalar.add(qkT_sb[cc][:], ps[:], bqk[:, cc:cc + 1])

                for vt in range(5):
                    n = TOK_SIZES[vt]
                    ps = ps_proj.tile([128, C], F32, tag="vps")
                    for ci in range(NCI):
                        nc.tensor.matmul(
                            ps[:n, :],
                            xT_sb[ci][:, 128 * vt:128 * vt + n],
                            w_sb[ci][:, 2 * C:3 * C],
                            start=(ci == 0),
                            stop=False,
                        )
                    nc.tensor.matmul(
                        ps[:n, :], ones[:, :n], bv[:], start=False, stop=True
                    )
                    nc.scalar.copy(v_sb[vt][:], ps[:n, :])

            # ---- phase 2: banded attention + output projection ----
            with (
                tc.tile_pool(name="ps_s", bufs=2, space="PSUM") as ps_s,
                tc.tile_pool(name="ps_pt", bufs=1, space="PSUM") as ps_pt,
                tc.tile_pool(name="ps_ot", bufs=1, space="PSUM") as ps_ot,
                tc.tile_pool(name="ps_y", bufs=2, space="PSUM") as ps_y,
            ):
                for kt in range(NKT):
                    vmask = 0 if kt == 0 else 1
                    q0 = W + 128 * kt   # local index of first q token of tile
                    j0 = 128 * kt       # local index of first window key
                    y_ps = ps_y.tile([128, C], F32, tag="y")
                    for pair in range(NPAIR):
                        qt = qkT_sb[pair]
                        ktile = qkT_sb[4 + pair]
                        s2 = [ps_s.tile([128, WIN], F32, tag=f"s2_{hh}",
                                        name=f"s2_{hh}")
                              for hh in range(2)]
                        for hh in range(2):
                            nc.tensor.matmul(
                                s2[hh][:],
                                qt[64 * hh:64 * (hh + 1), q0:q0 + 128],
                                ktile[64 * hh:64 * (hh + 1), j0:j0 + WIN],
                                start=True,
                                stop=True,
                            )
                        ms = wk.tile([128, 2 * WIN], F32, tag="ms")
                        pe = wk.tile([128, 2 * WIN], F32, tag="pe")
                        pn = wk.tile([128, 2 * WIN], F32, tag="pn")
                        zs = wk.tile([128, 2], F32, tag="zs")
                        ri = wk.tile([128, 2], F32, tag="ri")
                        for hh in range(2):
                            nc.vector.tensor_add(
                                ms[:, WIN * hh:WIN * (hh + 1)],
                                s2[hh][:],
                                mk_sb[vmask][:, WIN * hh:WIN * (hh + 1)],
                            )
                        for hh in range(2):
                            nc.scalar.activation(
                                pe[:, WIN * hh:WIN * (hh + 1)],
                                ms[:, WIN * hh:WIN * (hh + 1)],
                                mybir.ActivationFunctionType.Exp,
                                scale=1.0 / math.sqrt(HS),
                                accum_out=zs[:, hh:hh + 1],
                            )
                        nc.vector.reciprocal(ri[:], zs[:])
                        for hh in range(2):
                            c0 = WIN * hh
                            nc.scalar.activation(
                                pn[:, c0:c0 + WIN],
                                pe[:, c0:c0 + WIN],
                                mybir.ActivationFunctionType.Copy,
                                scale=ri[:, hh:hh + 1],
                            )
                        nc.sync.dma_start(
                            out=at_d[kt, 2 * pair:2 * pair + 2].rearrange(
                                "h i n -> i h n"
                            ),
                            in_=pn[:].rearrange("i (h n) -> i h n", h=2),
                        )
                        ots = wk.tile([128, 128], F32, tag="ots")
                        for hh in range(2):
                            h = 2 * pair + hh
                            c0 = WIN * hh
                            ptp = ps_pt.tile([128, 256], F32, tag="ptp")
                            nc.tensor.transpose(
                                ptp[:, 0:128], pn[:, c0:c0 + 128], ident[:]
                            )
                            nc.tensor.transpose(
                                ptp[0:W, 128:256],
                                pn[:, c0 + 128:c0 + WIN],
                                ident[:],
                            )
                            pts = wk.tile([128, 256], F32, tag="pts")
                            nc.vector.tensor_copy(pts[:, 0:128], ptp[:, 0:128])
                            nc.vector.tensor_copy(
                                pts[0:W, 128:256], ptp[0:W, 128:256]
                            )
                            ot = ps_ot.tile([64, 128], F32, tag="ot")
                            nc.tensor.matmul(
                                ot[:],
                                v_sb[kt][:, HS * h:HS * (h + 1)],
                                pts[:, 0:128],
                                start=True,
                                stop=False,
                            )
                            nc.tensor.matmul(
                                ot[:],
                                v_sb[kt + 1][0:W, HS * h:HS * (h + 1)],
                                pts[0:W, 128:256],
                                start=False,
                                stop=True,
                            )
                            nc.vector.tensor_copy(
                                ots[64 * hh:64 * (hh + 1), :], ot[:]
                            )
                        nc.tensor.matmul(
                            y_ps[:],
                            ots[:],
                            wp_sb[pair][:],
                            start=(pair == 0),
                            stop=False,
                        )
                    nc.tensor.matmul(
                        y_ps[:], ones[:, 0:128], bp[:], start=False, stop=True
                    )
                    y_sb = wk.tile([128, C], F32, tag="ysb")
                    nc.scalar.copy(y_sb[:], y_ps[:])
                    nc.sync.dma_start(
                        out=y_d[128 * kt:128 * (kt + 1), :], in_=y_sb[:]
                    )

    nc.compile()
    return nc


def make_masks():
    """masks[v, i(128), hh(2)*n(144)]: 0 in band, -1e30 outside.
    v=0: first tile of a shard (window cols < W are halo padding),
    v=1: normal tile.  Band: n in [i, i+W]; v=0 additionally n >= W."""
    i = np.arange(128)[:, None]
    n = np.arange(WIN)[None, :]
    band = (n >= i) & (n <= i + W)
    m1 = np.where(band, 0.0, -1e30).astype(np.float32)
    m0 = np.where(band & (n >= W), 0.0, -1e30).astype(np.float32)
    masks = np.zeros((2, 128, 2 * WIN), np.float32)
    masks[0] = np.concatenate([m0, m0], axis=1)
    masks[1] = np.concatenate([m1, m1], axis=1)
    return masks


_NC_CACHE = None


def _get_program():
    global _NC_CACHE
    if _NC_CACHE is None:
        _NC_CACHE = build_program()
    return _NC_CACHE


def make_in_maps(x, w_attn, b_attn, w_proj, b_proj):
    x = np.ascontiguousarray(np.asarray(x, np.float32))
    w_attn = np.ascontiguousarray(np.asarray(w_attn, np.float32))
    b_attn = np.asarray(b_attn, np.float32)
    w_proj = np.ascontiguousarray(np.asarray(w_proj, np.float32))
    b_proj = np.asarray(b_proj, np.float32)

    masks = make_masks()
    masks_mid = np.ascontiguousarray(np.stack([masks[1], masks[1]]))
    b_qk = np.ascontiguousarray(b_attn[:2 * C].reshape(8, 128).T)
    b_v = np.ascontiguousarray(b_attn[2 * C:].reshape(1, C))
    b_p = np.ascontiguousarray(b_proj.reshape(1, C))

    in_maps = []
    for core in range(NC):
        b, r = divmod(core, NC // B)
        t0 = r * TS
        x_shard = np.zeros((LT, C), np.float32)
        if r == 0:
            x_shard[W:] = x[b, t0:t0 + TS]
        else:
            x_shard[:] = x[b, t0 - W:t0 + TS]
        in_maps.append({
            "xT": np.ascontiguousarray(x_shard.T),
            "w_attn": w_attn,
            "b_qk": b_qk,
            "b_v": b_v,
            "w_proj": w_proj,
            "b_proj": b_p,
            "masks": masks if r == 0 else masks_mid,
        })
    return in_maps


def assemble(results):
    y_full = np.empty((B, T, C), np.float32)
    attn_full = np.zeros((B, H, T, T), np.float32)
    for core in range(NC):
        b, r = divmod(core, NC // B)
        t0 = r * TS
        y_full[b, t0:t0 + TS] = results[core]["y"]
        at = results[core]["attn_t"]          # (NKT, H, 128, WIN)
        for kt in range(NKT):
            i0 = t0 + 128 * kt
            j0 = t0 + 128 * kt - W
            blk = at[kt]                      # (H, 128, WIN)
            if j0 < 0:
                attn_full[b, :, i0:i0 + 128, 0:j0 + WIN] = blk[:, :, -j0:]
            else:
                attn_full[b, :, i0:i0 + 128, j0:j0 + WIN] = blk
    return y_full, attn_full


def kernel(x, w_attn, b_attn, w_proj, b_proj):
    in_maps = make_in_maps(x, w_attn, b_attn, w_proj, b_proj)
    nc = _get_program()
    res = run_bass_kernel_spmd(nc, in_maps, core_ids=list(range(NC)))
    return assemble(res.results)


# revision 23
# speedup vs baseline: 1.0835x; 1.0835x over previous
"""Trainium2 Bass kernel for banded (local-window) multi-head attention.

Reference computes, for x (B=2, T=2048, C=512):
    qkv = x @ w_attn + b_attn ; split into per-head q, k, v (H=8, hs=64)
    scores = q @ k^T / sqrt(hs), masked to the band j in [i-16, i]
    attn = softmax(scores)            -> (B, H, T, T), zero outside band
    y    = (attn @ v) @ w_proj + b_proj
returns (y, attn).

Sharding: data-parallel over (batch, token-quarter) -> 8 cores, each
handling 512 consecutive tokens of one batch with a 16-token halo of
x for the banded keys/values.  Each core computes the full qkv
projection for its shard, the banded softmax, and its slice of the
projected output.  The device emits attn as dense (128 x 144) window
tiles (zero outside the band by construction); the host scatters them
into the big, mostly-zero (B, H, T, T) output.
"""

import math

import numpy as np

import concourse.mybir as mybir
import concourse.tile as tile
from concourse import bacc
from concourse.bass_utils import run_bass_kernel_spmd
from concourse.masks import make_identity

F32 = mybir.dt.float32

B, T, C = 2, 2048, 512
H, HS = 8, 64
W = 16              # attention window (j in [i-16, i])
NC = 8              # cores
TS = T * B // NC    # tokens per shard = 512
LT = TS + W         # local tokens incl. halo = 528
NKT = TS // 128     # 128-token q tiles per shard = 4
WIN = 128 + W       # key window per q tile = 144
NCI = C // 128      # contraction chunks for projections = 4
NPAIR = H // 2      # head pairs = 4
TOK_SIZES = (128, 128, 128, 128, W)


def build_program():
    nc = bacc.Bacc("TRN2", target_bir_lowering=False, debug=False)

    xt_d = nc.dram_tensor("xT", [C, LT], F32, kind="ExternalInput").ap()
    wa_d = nc.dram_tensor("w_attn", [C, 3 * C], F32, kind="ExternalInput").ap()
    bqk_d = nc.dram_tensor("b_qk", [128, 8], F32, kind="ExternalInput").ap()
    wp_d = nc.dram_tensor("w_proj", [C, C], F32, kind="ExternalInput").ap()
    mk_d = nc.dram_tensor("masks", [2, 128, 2 * WIN], F32, kind="ExternalInput").ap()

    y_d = nc.dram_tensor("y", [TS, C], F32, kind="ExternalOutput").ap()
    at_d = nc.dram_tensor("attn_t", [NKT, 128, H, WIN], F32, kind="ExternalOutput").ap()

    with tile.TileContext(nc) as tc:
        with (
            tc.tile_pool(name="persist", bufs=1) as pp,
            tc.tile_pool(name="work", bufs=2) as wk,
        ):
            # ---- constants / parameters ----
            ident = pp.tile([128, 128], F32, tag="ident")
            make_identity(nc, ident[:])
            w_sb = []
            for ci in range(NCI):
                t = pp.tile([128, 3 * C], F32, tag=f"w{ci}")
                nc.sync.dma_start(out=t[:], in_=wa_d[128 * ci:128 * (ci + 1), :])
                w_sb.append(t)
            wp_sb = []
            for pr in range(NPAIR):
                t = pp.tile([128, C], F32, tag=f"wp{pr}")
                nc.sync.dma_start(out=t[:], in_=wp_d[128 * pr:128 * (pr + 1), :])
                wp_sb.append(t)
            bqk = pp.tile([128, 8], F32, tag="bqk")
            nc.sync.dma_start(out=bqk[:], in_=bqk_d[:])
            mk_sb = []
            for v in range(2):
                t = pp.tile([128, 2 * WIN], F32, tag=f"mask{v}")
                nc.sync.dma_start(out=t[:], in_=mk_d[v])
                mk_sb.append(t)

            xT_sb = [pp.tile([128, LT], F32, tag=f"xT{ci}", name=f"xT{ci}")
                     for ci in range(NCI)]
            # chunk cc in 0..3 -> q head-pair cc ; cc in 4..7 -> k head-pair cc-4
            qkT_sb = [pp.tile([128, LT], F32, tag=f"qkT{cc}", name=f"qkT{cc}")
                      for cc in range(8)]
            v_sb = [pp.tile([n, C], F32, tag=f"v{vt}", name=f"v{vt}")
                    for vt, n in enumerate(TOK_SIZES)]

            # ---- phase 1: load xT, project qkT and v ----
            with (
                tc.tile_pool(name="ps_proj", bufs=2, space="PSUM") as ps_proj,
            ):
                for ci in range(NCI):
                    nc.sync.dma_start(
                        out=xT_sb[ci][:],
                        in_=xt_d[128 * ci:128 * (ci + 1), :],
                    )

                for cc in range(8):
                    ps = ps_proj.tile([128, LT], F32, tag="qkvps")
                    for ci in range(NCI):
                        for n0, nn in ((0, 512), (512, LT - 512)):
                            nc.tensor.matmul(
                                ps[:, n0:n0 + nn],
                                w_sb[ci][:, 128 * cc:128 * (cc + 1)],
                                xT_sb[ci][:, n0:n0 + nn],
                                start=(ci == 0),
                                stop=(ci == NCI - 1),
                            )
                    nc.scalar.add(qkT_sb[cc][:], ps[:], bqk[:, cc:cc + 1])

                for vt in range(5):
                    n = TOK_SIZES[vt]
                    ps = ps_proj.tile([128, C], F32, tag="vps")
                    for ci in range(NCI):
                        nc.tensor.matmul(
                            ps[:n, :],
                            xT_sb[ci][:, 128 * vt:128 * vt + n],
                            w_sb[ci][:, 2 * C:3 * C],
                            start=(ci == 0),
                            stop=(ci == NCI - 1),
                        )
                    nc.scalar.copy(v_sb[vt][:], ps[:n, :])

            # ---- phase 2: banded attention + output projection ----
            with (
                tc.tile_pool(name="ps_s", bufs=2, space="PSUM") as ps_s,
                tc.tile_pool(name="ps_pt", bufs=1, space="PSUM") as ps_pt,
                tc.tile_pool(name="ps_ot", bufs=2, space="PSUM") as ps_ot,
                tc.tile_pool(name="ps_y", bufs=1, space="PSUM") as ps_y,
            ):
                for kt in range(NKT):
                    vmask = 0 if kt == 0 else 1
                    q0 = W + 128 * kt   # local index of first q token of tile
                    j0 = 128 * kt       # local index of first window key
                    y_ps = ps_y.tile([128, C], F32, tag="y")
                    for pair in range(NPAIR):
                        qt = qkT_sb[pair]
                        ktile = qkT_sb[4 + pair]
                        s2 = [ps_s.tile([128, WIN], F32, tag=f"s2_{hh}",
                                        name=f"s2_{hh}")
                              for hh in range(2)]
                        for hh in range(2):
                            # NOTE: the two heads' matmuls run concurrently
                            # (row groups 0/64) -- they must land in separate
                            # PSUM banks or the device faults.
                            nc.tensor.matmul(
                                s2[hh][:],
                                qt[64 * hh:64 * (hh + 1), q0:q0 + 128],
                                ktile[64 * hh:64 * (hh + 1), j0:j0 + WIN],
                                start=True,
                                stop=True,
                            )
                        ms = wk.tile([128, 2 * WIN], F32, tag="ms")
                        pe = wk.tile([128, 2 * WIN], F32, tag="pe")
                        pn = wk.tile([128, 2 * WIN], F32, tag="pn", bufs=4)
                        zs = wk.tile([128, 2], F32, tag="zs")
                        ri = wk.tile([128, 2], F32, tag="ri")
                        for hh in range(2):
                            nc.vector.tensor_add(
                                ms[:, WIN * hh:WIN * (hh + 1)],
                                s2[hh][:],
                                mk_sb[vmask][:, WIN * hh:WIN * (hh + 1)],
                            )
                        for hh in range(2):
                            nc.scalar.activation(
                                pe[:, WIN * hh:WIN * (hh + 1)],
                                ms[:, WIN * hh:WIN * (hh + 1)],
                                mybir.ActivationFunctionType.Exp,
                                scale=1.0 / math.sqrt(HS),
                                accum_out=zs[:, hh:hh + 1],
                            )
                        nc.vector.reciprocal(ri[:], zs[:])
                        for hh in range(2):
                            c0 = WIN * hh
                            nc.scalar.activation(
                                pn[:, c0:c0 + WIN],
                                pe[:, c0:c0 + WIN],
                                mybir.ActivationFunctionType.Copy,
                                scale=ri[:, hh:hh + 1],
                            )
                        nc.sync.dma_start(
                            out=at_d[kt][:, 2 * pair:2 * pair + 2, :],
                            in_=pn[:].rearrange("i (h n) -> i h n", h=2),
                        )
                        ots = wk.tile([128, 128], F32, tag="ots")
                        for hh in range(2):
                            h = 2 * pair + hh
                            c0 = WIN * hh
                            # pT0: full 128-key window transpose; the last 16
                            # keys (cols 128:144) only touch q rows >= 112, so
                            # only their 16x16 corner (i in [112,128)) is
                            # nonzero -- transpose/apply just that corner.
                            ptp = ps_pt.tile([128, 256], F32, tag="ptp")
                            nc.tensor.transpose(
                                ptp[:, 0:128], pn[:, c0:c0 + 128], ident[:]
                            )
                            nc.tensor.transpose(
                                ptp[0:W, 128:256],
                                pn[:, c0 + 128:c0 + WIN],
                                ident[:],
                            )
                            pts = wk.tile([128, 256], F32, tag="pts")
                            nc.vector.tensor_copy(pts[:, 0:128], ptp[:, 0:128])
                            nc.vector.tensor_copy(
                                pts[0:W, 240:256], ptp[0:W, 240:256]
                            )
                            ot = ps_ot.tile([64, 128], F32, tag="ot")
                            nc.tensor.matmul(
                                ot[:],
                                v_sb[kt][:, HS * h:HS * (h + 1)],
                                pts[:, 0:128],
                                start=True,
                                stop=False,
                            )
                            nc.tensor.matmul(
                                ot[:, 112:128],
                                v_sb[kt + 1][0:W, HS * h:HS * (h + 1)],
                                pts[0:W, 240:256],
                                start=False,
                                stop=True,
                            )
                            nc.vector.tensor_copy(
                                ots[64 * hh:64 * (hh + 1), :], ot[:]
                            )
                        nc.tensor.matmul(
                            y_ps[:],
                            ots[:],
                            wp_sb[pair][:],
                            start=(pair == 0),
                            stop=(pair == NPAIR - 1),
                        )
                    y_sb = wk.tile([128, C], F32, tag="ysb")
                    nc.scalar.copy(y_sb[:], y_ps[:])
                    nc.sync.dma_start(
                        out=y_d[128 * kt:128 * (kt + 1), :], in_=y_sb[:]
                    )

    nc.compile()
    return nc


def make_masks():
    """masks[v, i(128), hh(2)*n(144)]: 0 in band, -1e30 outside.
    v=0: first tile of a shard (window cols < W are halo padding),
    v=1: normal tile.  Band: n in [i, i+W]; v=0 additionally n >= W."""
    i = np.arange(128)[:, None]
    n = np.arange(WIN)[None, :]
    band = (n >= i) & (n <= i + W)
    m1 = np.where(band, 0.0, -1e30).astype(np.float32)
    m0 = np.where(band & (n >= W), 0.0, -1e30).astype(np.float32)
    masks = np.zeros((2, 128, 2 * WIN), np.float32)
    masks[0] = np.concatenate([m0, m0], axis=1)
    masks[1] = np.concatenate([m1, m1], axis=1)
    return masks


_NC_CACHE = None


def _get_program():
    global _NC_CACHE
    if _NC_CACHE is None:
        _NC_CACHE = build_program()
    return _NC_CACHE


def make_in_maps(x, w_attn, b_attn, w_proj, b_proj):
    x = np.ascontiguousarray(np.asarray(x, np.float32))
    w_attn = np.ascontiguousarray(np.asarray(w_attn, np.float32))
    b_attn = np.asarray(b_attn, np.float32)
    w_proj = np.ascontiguousarray(np.asarray(w_proj, np.float32))
    b_proj = np.asarray(b_proj, np.float32)

    masks = make_masks()
    masks_mid = np.ascontiguousarray(np.stack([masks[1], masks[1]]))
    b_qk = np.ascontiguousarray(b_attn[:2 * C].reshape(8, 128).T)
    b_v = np.ascontiguousarray(b_attn[2 * C:].reshape(1, C))
    b_p = np.ascontiguousarray(b_proj.reshape(1, C))

    in_maps = []
    for core in range(NC):
        b, r = divmod(core, NC // B)
        t0 = r * TS
        x_shard = np.zeros((LT, C), np.float32)
        if r == 0:
            x_shard[W:] = x[b, t0:t0 + TS]
        else:
            x_shard[:] = x[b, t0 - W:t0 + TS]
        in_maps.append({
            "xT": np.ascontiguousarray(x_shard.T),
            "w_attn": w_attn,
            "b_qk": b_qk,
            "w_proj": w_proj,
            "masks": masks if r == 0 else masks_mid,
        })
    # constant row folded into y on the host: softmax rows sum to 1, so the
    # v-bias contributes exactly b_v @ w_proj to every output row.
    y_const = (b_v @ w_proj + b_p).astype(np.float32)[0]
    return in_maps, y_const


def assemble(results, y_const):
    y_full = np.empty((B, T, C), np.float32)
    attn_full = np.zeros((B, H, T, T), np.float32)
    for core in range(NC):
        b, r = divmod(core, NC // B)
        t0 = r * TS
        y_full[b, t0:t0 + TS] = results[core]["y"]
        at = results[core]["attn_t"]          # (NKT, 128, H, WIN)
        for kt in range(NKT):
            i0 = t0 + 128 * kt
            j0 = t0 + 128 * kt - W
            blk = at[kt].transpose(1, 0, 2)   # (H, 128, WIN)
            if j0 < 0:
                attn_full[b, :, i0:i0 + 128, 0:j0 + WIN] = blk[:, :, -j0:]
            else:
                attn_full[b, :, i0:i0 + 128, j0:j0 + WIN] = blk
    if y_const.any():
        y_full += y_const
    return y_full, attn_full


def kernel(x, w_attn, b_attn, w_proj, b_proj):
    in_maps, y_const = make_in_maps(x, w_attn, b_attn, w_proj, b_proj)
    nc = _get_program()
    res = run_bass_kernel_spmd(nc, in_maps, core_ids=list(range(NC)))
    return assemble(res.results, y_const)


# revision 26
# speedup vs baseline: 1.1856x; 1.0942x over previous
"""Trainium2 Bass kernel for banded (local-window) multi-head attention.

Reference computes, for x (B=2, T=2048, C=512):
    qkv = x @ w_attn + b_attn ; split into per-head q, k, v (H=8, hs=64)
    scores = q @ k^T / sqrt(hs), masked to the band j in [i-16, i]
    attn = softmax(scores)            -> (B, H, T, T), zero outside band
    y    = (attn @ v) @ w_proj + b_proj
returns (y, attn).

Sharding: data-parallel over (batch, token-quarter) -> 8 cores, each
handling 512 consecutive tokens of one batch with a 16-token halo of
x for the banded keys/values.  Each core computes the full qkv
projection for its shard, the banded softmax, and its slice of the
projected output.  The device emits attn as dense (128 x 144) window
tiles (zero outside the band by construction); the host scatters them
into the big, mostly-zero (B, H, T, T) output.
"""

import math

import numpy as np

import concourse.mybir as mybir
import concourse.tile as tile
from concourse import bacc
from concourse.bass_utils import run_bass_kernel_spmd
from concourse.masks import make_identity

F32 = mybir.dt.float32

B, T, C = 2, 2048, 512
H, HS = 8, 64
W = 16              # attention window (j in [i-16, i])
NC = 8              # cores
TS = T * B // NC    # tokens per shard = 512
LT = TS + W         # local tokens incl. halo = 528
NKT = TS // 128     # 128-token q tiles per shard = 4
WIN = 128 + W       # key window per q tile = 144
NCI = C // 128      # contraction chunks for projections = 4
NPAIR = H // 2      # head pairs = 4
TOK_SIZES = (128, 128, 128, 128, W)


def build_program():
    nc = bacc.Bacc("TRN2", target_bir_lowering=False, debug=False)

    xt_d = nc.dram_tensor("xT", [C, LT], F32, kind="ExternalInput").ap()
    wa_d = nc.dram_tensor("w_attn", [C, 3 * C], F32, kind="ExternalInput").ap()
    bqk_d = nc.dram_tensor("b_qk", [128, 8], F32, kind="ExternalInput").ap()
    wp_d = nc.dram_tensor("w_proj", [C, C], F32, kind="ExternalInput").ap()
    mk_d = nc.dram_tensor("masks", [2, 128, 2 * WIN], F32, kind="ExternalInput").ap()

    y_d = nc.dram_tensor("y", [TS, C], F32, kind="ExternalOutput").ap()
    at_d = nc.dram_tensor("attn_t", [NKT, 128, H, WIN], F32, kind="ExternalOutput").ap()

    with tile.TileContext(nc) as tc:
        with (
            tc.tile_pool(name="persist", bufs=1) as pp,
            tc.tile_pool(name="work", bufs=2) as wk,
        ):
            # ---- constants / parameters ----
            ident = pp.tile([128, 128], F32, tag="ident")
            make_identity(nc, ident[:])
            w_sb = []
            for ci in range(NCI):
                t = pp.tile([128, 3 * C], F32, tag=f"w{ci}")
                nc.sync.dma_start(out=t[:], in_=wa_d[128 * ci:128 * (ci + 1), :])
                w_sb.append(t)
            wp_sb = []
            for pr in range(NPAIR):
                t = pp.tile([128, C], F32, tag=f"wp{pr}")
                nc.sync.dma_start(out=t[:], in_=wp_d[128 * pr:128 * (pr + 1), :])
                wp_sb.append(t)
            bqk = pp.tile([128, 8], F32, tag="bqk")
            nc.sync.dma_start(out=bqk[:], in_=bqk_d[:])
            mk_sb = []
            for v in range(2):
                t = pp.tile([128, 2 * WIN], F32, tag=f"mask{v}")
                nc.sync.dma_start(out=t[:], in_=mk_d[v])
                mk_sb.append(t)

            xT_sb = [pp.tile([128, LT], F32, tag=f"xT{ci}", name=f"xT{ci}")
                     for ci in range(NCI)]
            # chunk cc in 0..3 -> q head-pair cc ; cc in 4..7 -> k head-pair cc-4
            qkT_sb = [pp.tile([128, LT], F32, tag=f"qkT{cc}", name=f"qkT{cc}")
                      for cc in range(8)]
            v_sb = [pp.tile([n, C], F32, tag=f"v{vt}", name=f"v{vt}")
                    for vt, n in enumerate(TOK_SIZES)]

            # ---- phase 1: load xT, project qkT and v ----
            with (
                tc.tile_pool(name="ps_proj", bufs=2, space="PSUM") as ps_proj,
            ):
                for ci in range(NCI):
                    nc.sync.dma_start(
                        out=xT_sb[ci][:],
                        in_=xt_d[128 * ci:128 * (ci + 1), :],
                    )

                for cc in range(8):
                    ps = ps_proj.tile([128, LT], F32, tag="qkvps")
                    for ci in range(NCI):
                        for n0, nn in ((0, 512), (512, LT - 512)):
                            nc.tensor.matmul(
                                ps[:, n0:n0 + nn],
                                w_sb[ci][:, 128 * cc:128 * (cc + 1)],
                                xT_sb[ci][:, n0:n0 + nn],
                                start=(ci == 0),
                                stop=(ci == NCI - 1),
                            )
                    nc.scalar.add(qkT_sb[cc][:], ps[:], bqk[:, cc:cc + 1])

                for vt in range(5):
                    n = TOK_SIZES[vt]
                    ps = ps_proj.tile([128, C], F32, tag="vps")
                    for ci in range(NCI):
                        nc.tensor.matmul(
                            ps[:n, :],
                            xT_sb[ci][:, 128 * vt:128 * vt + n],
                            w_sb[ci][:, 2 * C:3 * C],
                            start=(ci == 0),
                            stop=(ci == NCI - 1),
                        )
                    nc.scalar.copy(v_sb[vt][:], ps[:n, :])

            # ---- phase 2: banded attention + output projection ----
            with (
                tc.tile_pool(name="ps_s", bufs=2, space="PSUM") as ps_s,
                tc.tile_pool(name="ps_pt", bufs=2, space="PSUM") as ps_pt,
                tc.tile_pool(name="ps_ot", bufs=2, space="PSUM") as ps_ot,
                tc.tile_pool(name="ps_y", bufs=2, space="PSUM") as ps_y,
            ):
                for kt in range(NKT):
                    vmask = 0 if kt == 0 else 1
                    q0 = W + 128 * kt   # local index of first q token of tile
                    j0 = 128 * kt       # local index of first window key
                    y_ps = ps_y.tile([128, C], F32, tag="y")
                    for pair in range(NPAIR):
                        qt = qkT_sb[pair]
                        ktile = qkT_sb[4 + pair]
                        s2 = [ps_s.tile([128, WIN], F32, tag=f"s2_{hh}",
                                        name=f"s2_{hh}", bufs=1)
                              for hh in range(2)]
                        for hh in range(2):
                            # NOTE: the two heads' matmuls run concurrently
                            # (row groups 0/64) -- they must land in separate
                            # PSUM banks or the device faults.
                            nc.tensor.matmul(
                                s2[hh][:],
                                qt[64 * hh:64 * (hh + 1), q0:q0 + 128],
                                ktile[64 * hh:64 * (hh + 1), j0:j0 + WIN],
                                start=True,
                                stop=True,
                            )
                        ms = wk.tile([128, 2 * WIN], F32, tag="ms")
                        pe = wk.tile([128, 2 * WIN], F32, tag="pe")
                        pn = wk.tile([128, 2 * WIN], F32, tag="pn", bufs=4)
                        zs = wk.tile([128, 2], F32, tag="zs")
                        ri = wk.tile([128, 2], F32, tag="ri")
                        for hh in range(2):
                            nc.vector.tensor_add(
                                ms[:, WIN * hh:WIN * (hh + 1)],
                                s2[hh][:],
                                mk_sb[vmask][:, WIN * hh:WIN * (hh + 1)],
                            )
                        for hh in range(2):
                            nc.scalar.activation(
                                pe[:, WIN * hh:WIN * (hh + 1)],
                                ms[:, WIN * hh:WIN * (hh + 1)],
                                mybir.ActivationFunctionType.Exp,
                                scale=1.0 / math.sqrt(HS),
                                accum_out=zs[:, hh:hh + 1],
                            )
                        nc.vector.reciprocal(ri[:], zs[:])
                        for hh in range(2):
                            c0 = WIN * hh
                            nc.scalar.activation(
                                pn[:, c0:c0 + WIN],
                                pe[:, c0:c0 + WIN],
                                mybir.ActivationFunctionType.Copy,
                                scale=ri[:, hh:hh + 1],
                            )
                        nc.sync.dma_start(
                            out=at_d[kt][:, 2 * pair:2 * pair + 2, :],
                            in_=pn[:].rearrange("i (h n) -> i h n", h=2),
                        )
                        ots = wk.tile([128, 128], F32, tag="ots")
                        for hh in range(2):
                            h = 2 * pair + hh
                            c0 = WIN * hh
                            # pT0: full 128-key window transpose; the last 16
                            # keys (cols 128:144) only touch q rows >= 112, so
                            # only their 16x16 corner (i in [112,128)) is
                            # nonzero -- transpose/apply just that corner.
                            ptp = ps_pt.tile([128, 256], F32, tag="ptp")
                            nc.tensor.transpose(
                                ptp[:, 0:128], pn[:, c0:c0 + 128], ident[:]
                            )
                            nc.tensor.transpose(
                                ptp[0:W, 128:256],
                                pn[:, c0 + 128:c0 + WIN],
                                ident[:],
                            )
                            pts = wk.tile([128, 256], F32, tag="pts")
                            nc.vector.tensor_copy(pts[:, 0:128], ptp[:, 0:128])
                            nc.vector.tensor_copy(
                                pts[0:W, 240:256], ptp[0:W, 240:256]
                            )
                            ot = ps_ot.tile([64, 128], F32, tag="ot")
                            nc.tensor.matmul(
                                ot[:],
                                v_sb[kt][:, HS * h:HS * (h + 1)],
                                pts[:, 0:128],
                                start=True,
                                stop=False,
                            )
                            nc.tensor.matmul(
                                ot[:, 112:128],
                                v_sb[kt + 1][0:W, HS * h:HS * (h + 1)],
                                pts[0:W, 240:256],
                                start=False,
                                stop=True,
                            )
                            nc.vector.tensor_copy(
                                ots[64 * hh:64 * (hh + 1), :], ot[:]
                            )
                        nc.tensor.matmul(
                            y_ps[:],
                            ots[:],
                            wp_sb[pair][:],
                            start=(pair == 0),
                            stop=(pair == NPAIR - 1),
                        )
                    y_sb = wk.tile([128, C], F32, tag="ysb")
                    nc.scalar.copy(y_sb[:], y_ps[:])
                    nc.sync.dma_start(
                        out=y_d[128 * kt:128 * (kt + 1), :], in_=y_sb[:]
                    )

    nc.compile()
    return nc


def make_masks():
    """masks[v, i(128), hh(2)*n(144)]: 0 in band, -1e30 outside.
    v=0: first tile of a shard (window cols < W are halo padding),
    v=1: normal tile.  Band: n in [i, i+W]; v=0 additionally n >= W."""
    i = np.arange(128)[:, None]
    n = np.arange(WIN)[None, :]
    band = (n >= i) & (n <= i + W)
    m1 = np.where(band, 0.0, -1e30).astype(np.float32)
    m0 = np.where(band & (n >= W), 0.0, -1e30).astype(np.float32)
    masks = np.zeros((2, 128, 2 * WIN), np.float32)
    masks[0] = np.concatenate([m0, m0], axis=1)
    masks[1] = np.concatenate([m1, m1], axis=1)
    return masks


_NC_CACHE = None


def _get_program():
    global _NC_CACHE
    if _NC_CACHE is None:
        _NC_CACHE = build_program()
    return _NC_CACHE


def make_in_maps(x, w_attn, b_attn, w_proj, b_proj):
    x = np.ascontiguousarray(np.asarray(x, np.float32))
    w_attn = np.ascontiguousarray(np.asarray(w_attn, np.float32))
    b_attn = np.asarray(b_attn, np.float32)
    w_proj = np.ascontiguousarray(np.asarray(w_proj, np.float32))
    b_proj = np.asarray(b_proj, np.float32)

    masks = make_masks()
    masks_mid = np.ascontiguousarray(np.stack([masks[1], masks[1]]))
    b_qk = np.ascontiguousarray(b_attn[:2 * C].reshape(8, 128).T)
    b_v = np.ascontiguousarray(b_attn[2 * C:].reshape(1, C))
    b_p = np.ascontiguousarray(b_proj.reshape(1, C))

    in_maps = []
    for core in range(NC):
        b, r = divmod(core, NC // B)
        t0 = r * TS
        x_shard = np.zeros((LT, C), np.float32)
        if r == 0:
            x_shard[W:] = x[b, t0:t0 + TS]
        else:
            x_shard[:] = x[b, t0 - W:t0 + TS]
        in_maps.append({
            "xT": np.ascontiguousarray(x_shard.T),
            "w_attn": w_attn,
            "b_qk": b_qk,
            "w_proj": w_proj,
            "masks": masks if r == 0 else masks_mid,
        })
    # constant row folded into y on the host: softmax rows sum to 1, so the
    # v-bias contributes exactly b_v @ w_proj to every output row.
    y_const = (b_v @ w_proj + b_p).astype(np.float32)[0]
    return in_maps, y_const


def assemble(results, y_const):
    y_full = np.empty((B, T, C), np.float32)
    attn_full = np.zeros((B, H, T, T), np.float32)
    for core in range(NC):
        b, r = divmod(core, NC // B)
        t0 = r * TS
        y_full[b, t0:t0 + TS] = results[core]["y"]
        at = results[core]["attn_t"]          # (NKT, 128, H, WIN)
        for kt in range(NKT):
            i0 = t0 + 128 * kt
            j0 = t0 + 128 * kt - W
            blk = at[kt].transpose(1, 0, 2)   # (H, 128, WIN)
            if j0 < 0:
                attn_full[b, :, i0:i0 + 128, 0:j0 + WIN] = blk[:, :, -j0:]
            else:
                attn_full[b, :, i0:i0 + 128, j0:j0 + WIN] = blk
    if y_const.any():
        y_full += y_const
    return y_full, attn_full


def kernel(x, w_attn, b_attn, w_proj, b_proj):
    in_maps, y_const = make_in_maps(x, w_attn, b_attn, w_proj, b_proj)
    nc = _get_program()
    res = run_bass_kernel_spmd(nc, in_maps, core_ids=list(range(NC)))
    return assemble(res.results, y_const)
